# revision 11
# baseline (speedup 1.0000x reference)
"""GATv2Conv-with-edge-features Trainium2 kernel (8-core SPMD, edge-sharded by dst).

Self-contained: hardcodes problem shapes (N=50000 nodes, E=800000 edges,
128 feat, 8 heads x 16). Sharding: core k owns dst nodes [6250k, 6250(k+1))
and all edges pointing into that range.

v3: block-aligned tiling, single-phase program.
Edges sorted by dst are packed into tiles of <=128 edges; a tile never
crosses a 32-node block boundary; a super-tile (st) is 16 tiles spanning
<=4 consecutive blocks, i.e. a 128-node window [n0, n0+128). Per st:
  one fp16 DMA [x[src] | efeat | x[window].T] + one fp8 DMA [S | S^T]
  FDwin(psum) = xwin.T @ Wd  -> fp16 (ACT copy): feat_dst for the window
  T(psum)[e,hd] = xs@Ws + ef@We + S^T @ FDwin   (3 matmuls/tile; the S^T
     one-hot broadcasts per-slot feat_dst to edges, slot = node - n0)
  T16 = ACT copy; L = max(T16, 0.2*T16); LA = L*attn (DVE fp16 2x);
  score = tree-add over d (fp16 2x, no TensorReduce); ex = exp(score);
  msg = T16*ex (Pool engine, both halves);
  U(psum)[slot,f] = sum_t S_t.T @ msg_t ; z[slot,h] likewise (slot rows of
     ALL 16 tiles accumulate into one [128,128] psum = window node rows)
  H = relu(U*(z>0)/max(z,eps) - (z>0)*FDwin)  -> stored to H_d[st*128 ...]
Host: out[n0:n0+128] += H[st] per st (non-owned rows are exactly 0), so the
program is identical across cores (SPMD) despite per-core windows.
"""
import numpy as np
import ml_dtypes

import concourse.bacc as bacc
import concourse.bass as bass
import concourse.tile as tile
import concourse.mybir as mybir
from concourse.bass_utils import run_bass_kernel_spmd

N_NODES = 50000
N_CORES = 8
N_LOCAL = N_NODES // N_CORES          # 6250
IN_FEAT = 128
HEADS = 8
HEAD_DIM = 16
NEG_SLOPE = 0.2
TILE_E = 128                          # edges per tile
BLK = 32                              # nodes per aligned block
ST_TILES = 16                         # tiles per super-tile
ST_BLKS = 4                           # max blocks spanned by one st
WIN = BLK * ST_BLKS                   # 128-node window per st
EDGE_BLK = ST_TILES * TILE_E          # 2048 edge slots per st
MG_COLS = 2 * EDGE_BLK + WIN          # xs | ef | xwinT = 4224 fp16 cols
OFF_XS = 0
OFF_EF = EDGE_BLK
OFF_XW = 2 * EDGE_BLK
OH_COLS = 2 * EDGE_BLK                # S | S^T = 4096 fp8 cols
EPS_Z = 1e-12
P = 128
FP = mybir.dt.float32
BF = mybir.dt.float16
F8 = mybir.dt.float8e4
BF_NP = np.float16
F8_NP = ml_dtypes.float8_e4m3fn


# ---------------------------------------------------------------- host prep

def _pack_core(dst_sorted_local, deg):
    """Block-aligned tile packing. Returns per-tile (block, count) and the
    edge order is implicit (dst-sorted). A tile's edges all lie in one
    32-node block; a node's edges never split across tiles."""
    tiles = []                  # (block, n_edges)
    cur_blk = -1
    cur_cnt = 0
    for n in range(len(deg)):
        d = int(deg[n])
        if d == 0:
            continue
        assert d <= TILE_E, f"node degree {d} exceeds tile capacity {TILE_E}"
        b = n // BLK
        if b != cur_blk or cur_cnt + d > TILE_E:
            if cur_cnt > 0:
                tiles.append((cur_blk, cur_cnt))
            cur_blk, cur_cnt = b, 0
        cur_cnt += d
    if cur_cnt > 0:
        tiles.append((cur_blk, cur_cnt))

    # group tiles into super-tiles: <=16 tiles, <=4 blocks span
    sts = []                    # list of (a0, [tile indices])
    cur = []
    a0 = None
    for ti, (b, _) in enumerate(tiles):
        if cur and (len(cur) == ST_TILES or b - a0 >= ST_BLKS):
            sts.append((a0, cur))
            cur = []
        if not cur:
            a0 = b
        cur.append(ti)
    if cur:
        sts.append((a0, cur))
    return tiles, sts


def _prep_cores(x, efeat, src, dst, W_src, b_src, W_dst, b_dst, W_edge, attn):
    x = np.ascontiguousarray(np.asarray(x, np.float32))
    efeat = np.asarray(efeat, np.float32)
    src = np.asarray(src).astype(np.int64)
    dst = np.asarray(dst).astype(np.int64)
    W_src = np.asarray(W_src, np.float32)
    W_dst = np.asarray(W_dst, np.float32)
    W_edge = np.asarray(W_edge, np.float32)
    b_src = np.asarray(b_src, np.float32)
    b_dst = np.asarray(b_dst, np.float32)
    attn = np.asarray(attn, np.float32)
    has_bias = bool(max(np.abs(b_src).max(), np.abs(b_dst).max()) > 0)

    per_core = []
    core_nst = []
    for k in range(N_CORES):
        lo = k * N_LOCAL
        eidx = np.nonzero((dst >= lo) & (dst < lo + N_LOCAL))[0]
        dl = dst[eidx] - lo
        order = np.argsort(dl, kind="stable")
        eidx, dl = eidx[order], dl[order]
        deg = np.bincount(dl, minlength=N_LOCAL)
        tiles, sts = _pack_core(dl, deg)
        per_core.append((eidx, dl, tiles, sts))
        core_nst.append(len(sts))

    n_st = max(core_nst)
    x16 = x.astype(BF_NP)
    ef16 = efeat.astype(BF_NP)
    attn16 = np.ascontiguousarray(
        np.broadcast_to(attn.reshape(1, HEADS * HEAD_DIM),
                        (P, HEADS * HEAD_DIM)).astype(BF_NP))
    bsrc16 = np.ascontiguousarray(
        np.broadcast_to(b_src[None, :], (P, IN_FEAT)).astype(BF_NP))
    bdst16 = np.ascontiguousarray(
        np.broadcast_to(b_dst[None, :], (P, IN_FEAT)).astype(BF_NP))

    in_maps = []
    metas = []
    for k in range(N_CORES):
        eidx, dl, tiles, sts = per_core[k]
        # tile edge start offsets
        starts = np.zeros(len(tiles) + 1, np.int64)
        for ti, (_, c) in enumerate(tiles):
            starts[ti + 1] = starts[ti] + c

        mg = np.zeros((P, n_st * MG_COLS), BF_NP)
        oh = np.zeros((P, n_st * OH_COLS), F8_NP)
        n0s = np.zeros(n_st, np.int64)
        xpad = np.zeros((WIN, IN_FEAT), BF_NP)
        for st, (a0, tl_list) in enumerate(sts):
            n0 = a0 * BLK
            n0s[st] = n0
            b0 = st * MG_COLS
            o0 = st * OH_COLS
            # window x rows (transposed) for on-device feat_dst projection
            hi = min(n0 + WIN, N_LOCAL)
            xw = xpad.copy()
            xw[:hi - n0] = x16[k * N_LOCAL + n0:k * N_LOCAL + hi]
            mg[:, b0 + OFF_XW:b0 + OFF_XW + WIN] = xw.T
            for tl, ti in enumerate(tl_list):
                e_ids = eidx[starts[ti]:starts[ti + 1]]
                slot = (dl[starts[ti]:starts[ti + 1]] - n0).astype(np.int64)
                c = len(e_ids)
                ep = np.arange(c)
                col = b0 + tl * TILE_E
                mg[:, col + OFF_XS:col + OFF_XS + c] = x16[src[e_ids]].T
                mg[:, col + OFF_EF:col + OFF_EF + c] = ef16[e_ids].T
                oh[ep, o0 + tl * TILE_E + slot] = 1.0          # S [e, slot]
                oh[slot, o0 + EDGE_BLK + tl * TILE_E + ep] = 1.0  # S^T
        in_maps.append(dict(
            Ws16=np.ascontiguousarray(W_src.T.astype(BF_NP)),
            Wd16=np.ascontiguousarray(W_dst.T.astype(BF_NP)),
            We16=np.ascontiguousarray(W_edge.T.astype(BF_NP)),
            bsrc16=bsrc16,
            bdst16=bdst16,
            attn16=attn16,
            mg=np.ascontiguousarray(mg),
            oh=np.ascontiguousarray(oh),
        ))
        metas.append(n0s)
    return in_maps, metas, n_st, has_bias


# ------------------------------------------------------------- bass program

def build_program(n_st, has_bias=False):
    nc = bacc.Bacc("TRN2", target_bir_lowering=False, debug=False,
                   num_devices=N_CORES)

    Ws16_d = nc.dram_tensor("Ws16", [IN_FEAT, IN_FEAT], BF, kind="ExternalInput")
    Wd16_d = nc.dram_tensor("Wd16", [IN_FEAT, IN_FEAT], BF, kind="ExternalInput")
    We16_d = nc.dram_tensor("We16", [IN_FEAT, IN_FEAT], BF, kind="ExternalInput")
    bsrc_d = nc.dram_tensor("bsrc16", [P, IN_FEAT], BF, kind="ExternalInput")
    bdst_d = nc.dram_tensor("bdst16", [P, IN_FEAT], BF, kind="ExternalInput")
    attn_d = nc.dram_tensor("attn16", [P, IN_FEAT], BF, kind="ExternalInput")
    mg_d = nc.dram_tensor("mg", [P, n_st * MG_COLS], BF, kind="ExternalInput")
    oh_d = nc.dram_tensor("oh", [P, n_st * OH_COLS], F8, kind="ExternalInput")
    H_d = nc.dram_tensor("H", [n_st * WIN, IN_FEAT], FP, kind="ExternalOutput")

    with tile.TileContext(nc) as tc:
        with tc.tile_pool(name="const", bufs=1) as cb:
            Ws16 = cb.tile([P, IN_FEAT], BF)
            nc.sync.dma_start(out=Ws16[:], in_=Ws16_d[:])
            Wd16 = cb.tile([P, IN_FEAT], BF)
            nc.sync.dma_start(out=Wd16[:], in_=Wd16_d[:])
            We16 = cb.tile([P, IN_FEAT], BF)
            nc.sync.dma_start(out=We16[:], in_=We16_d[:])
            bsrc = cb.tile([P, IN_FEAT], BF)
            nc.sync.dma_start(out=bsrc[:], in_=bsrc_d[:])
            bdst = cb.tile([P, IN_FEAT], BF)
            nc.sync.dma_start(out=bdst[:], in_=bdst_d[:])
            attn_sb = cb.tile([P, IN_FEAT], BF)
            nc.sync.dma_start(out=attn_sb[:], in_=attn_d[:])

            with (
                tc.tile_pool(name="eb_sb", bufs=3) as eb,
                tc.tile_pool(name="eb_ps", bufs=2, space="PSUM") as ep,
                tc.tile_pool(name="eb_psU", bufs=2, space="PSUM") as epU,
                tc.tile_pool(name="eb_psz", bufs=2, space="PSUM") as epz,
            ):
                for st in range(n_st):
                    b0 = st * MG_COLS
                    o0 = st * OH_COLS
                    mg_sb = eb.tile([P, MG_COLS], BF, tag="mg")
                    nc.sync.dma_start(out=mg_sb[:], in_=mg_d[:, b0:b0 + MG_COLS])
                    oh_sb = eb.tile([P, OH_COLS], F8, tag="oh")
                    nc.sync.dma_start(out=oh_sb[:], in_=oh_d[:, o0:o0 + OH_COLS])

                    # window feat_dst = xwin.T @ Wd  (on device)
                    f_ps = epU.tile([P, IN_FEAT], FP, tag="Ups")
                    nc.tensor.matmul(out=f_ps[:BLK, :1],
                                     lhsT=mg_sb[:, :BLK], rhs=mg_sb[:, :1],
                                     start=True, stop=True,
                                     skip_group_check=True)
                    nc.tensor.matmul(out=f_ps[:BLK, :1],
                                     lhsT=oh_sb[:, :BLK], rhs=oh_sb[:, :1],
                                     start=True, stop=True,
                                     skip_group_check=True)
                    nc.tensor.matmul(out=f_ps[:],
                                     lhsT=mg_sb[:, OFF_XW:OFF_XW + WIN],
                                     rhs=Wd16[:], start=True, stop=True)
                    FDB = eb.tile([P, IN_FEAT], BF, tag="FDB")
                    nc.scalar.activation(out=FDB[:], in_=f_ps[:],
                                         func=mybir.ActivationFunctionType.Copy)
                    if has_bias:
                        nc.vector.tensor_tensor(out=FDB[:], in0=FDB[:],
                                                in1=bdst[:],
                                                op=mybir.AluOpType.add)

                    T16 = []
                    for h in range(2):
                        Th = ep.tile([P, 8 * IN_FEAT], FP, tag="T")
                        for tt in range(8):
                            t = h * 8 + tt
                            c = t * TILE_E
                            sl = slice(tt * IN_FEAT, (tt + 1) * IN_FEAT)
                            nc.tensor.matmul(
                                out=Th[:, sl],
                                lhsT=mg_sb[:, OFF_XS + c:OFF_XS + c + TILE_E],
                                rhs=Ws16[:], start=True, stop=False)
                            nc.tensor.matmul(
                                out=Th[:, sl],
                                lhsT=mg_sb[:, OFF_EF + c:OFF_EF + c + TILE_E],
                                rhs=We16[:], start=False, stop=False)
                            nc.tensor.matmul(
                                out=Th[:, sl],
                                lhsT=oh_sb[:, EDGE_BLK + c:
                                           EDGE_BLK + c + TILE_E],
                                rhs=FDB[:], start=False, stop=True)
                        T16h = eb.tile([P, 8 * IN_FEAT], BF, tag=f"T16{h}")
                        nc.scalar.activation(out=T16h[:], in_=Th[:],
                                             func=mybir.ActivationFunctionType.Copy)
                        if has_bias:
                            nc.vector.tensor_tensor(
                                out=T16h[:].rearrange("p (t f) -> p t f", t=8),
                                in0=T16h[:].rearrange("p (t f) -> p t f", t=8),
                                in1=bsrc[:].unsqueeze(1).to_broadcast(
                                    [P, 8, IN_FEAT]),
                                op=mybir.AluOpType.add)
                        T16.append(T16h)

                    # score chain (DVE, fp16 2x where possible)
                    score = eb.tile([P, ST_TILES * HEADS], BF, tag="score")
                    for h in range(2):
                        T16h = T16[h]
                        T2 = eb.tile([P, 8 * IN_FEAT], BF, tag="T2")
                        nc.vector.tensor_scalar(
                            out=T2[:], in0=T16h[:], scalar1=NEG_SLOPE,
                            scalar2=None, op0=mybir.AluOpType.mult)
                        L = eb.tile([P, 8 * IN_FEAT], BF, tag="L")
                        nc.vector.tensor_tensor(out=L[:], in0=T16h[:],
                                                in1=T2[:],
                                                op=mybir.AluOpType.max)
                        LA = eb.tile([P, 8 * IN_FEAT], BF, tag="LA")
                        nc.vector.tensor_tensor(
                            out=LA[:].rearrange("p (t f) -> p t f", t=8),
                            in0=L[:].rearrange("p (t f) -> p t f", t=8),
                            in1=attn_sb[:].unsqueeze(1).to_broadcast(
                                [P, 8, IN_FEAT]),
                            op=mybir.AluOpType.mult)
                        LAv = LA[:].rearrange("p (t hh d) -> p t hh d",
                                              hh=HEADS, d=HEAD_DIM)
                        s1 = eb.tile([P, 512], BF, tag="s1")
                        s1v = s1[:].rearrange("p (t hh d) -> p t hh d",
                                              hh=HEADS, d=8)
                        nc.vector.tensor_tensor(out=s1v, in0=LAv[:, :, :, :8],
                                                in1=LAv[:, :, :, 8:],
                                                op=mybir.AluOpType.add)
                        s2 = eb.tile([P, 256], BF, tag="s2")
                        s2v = s2[:].rearrange("p (t hh d) -> p t hh d",
                                              hh=HEADS, d=4)
                        nc.vector.tensor_tensor(out=s2v, in0=s1v[:, :, :, :4],
                                                in1=s1v[:, :, :, 4:],
                                                op=mybir.AluOpType.add)
                        s3 = eb.tile([P, 128], BF, tag="s3")
                        s3v = s3[:].rearrange("p (t hh d) -> p t hh d",
                                              hh=HEADS, d=2)
                        nc.vector.tensor_tensor(out=s3v, in0=s2v[:, :, :, :2],
                                                in1=s2v[:, :, :, 2:],
                                                op=mybir.AluOpType.add)
                        sc = score[:, h * 64:(h + 1) * 64].rearrange(
                            "p (t hh) -> p t hh", hh=HEADS).unsqueeze(3)
                        nc.vector.tensor_tensor(out=sc, in0=s3v[:, :, :, :1],
                                                in1=s3v[:, :, :, 1:],
                                                op=mybir.AluOpType.add)

                    ex = eb.tile([P, ST_TILES * HEADS], BF, tag="ex")
                    nc.scalar.activation(out=ex[:], in_=score[:],
                                         func=mybir.ActivationFunctionType.Exp)

                    # msg = T16 * ex (broadcast over d) on Pool engine
                    msg = []
                    for h in range(2):
                        m = eb.tile([P, 8 * IN_FEAT], BF, tag=f"msg{h}")
                        ex_b = ex[:, h * 64:(h + 1) * 64] \
                            .rearrange("p (t hh) -> p t hh", hh=HEADS) \
                            .unsqueeze(3).to_broadcast([P, 8, HEADS, HEAD_DIM])
                        nc.gpsimd.tensor_tensor(
                            out=m[:].rearrange("p (t hh d) -> p t hh d",
                                               hh=HEADS, d=HEAD_DIM),
                            in0=T16[h][:].rearrange("p (t hh d) -> p t hh d",
                                                    hh=HEADS, d=HEAD_DIM),
                            in1=ex_b, op=mybir.AluOpType.mult)
                        msg.append(m)

                    # scatter all 16 tiles into one window psum
                    U_ps = epU.tile([P, IN_FEAT], FP, tag="Ups")
                    z_ps = epz.tile([P, HEADS], FP, tag="zps")
                    nc.tensor.matmul(out=U_ps[:BLK, :1],
                                     lhsT=msg[1][:, :BLK], rhs=msg[1][:, :1],
                                     start=True, stop=True,
                                     skip_group_check=True)
                    for t in range(ST_TILES):
                        h, tloc = t // 8, t % 8
                        Scol = oh_sb[:, t * TILE_E:(t + 1) * TILE_E]
                        nc.tensor.matmul(
                            out=U_ps[:],
                            lhsT=Scol,
                            rhs=msg[h][:, tloc * IN_FEAT:(tloc + 1) * IN_FEAT],
                            start=(t == 0), stop=(t == ST_TILES - 1))
                        nc.tensor.matmul(
                            out=z_ps[:],
                            lhsT=Scol,
                            rhs=ex[:, t * HEADS:(t + 1) * HEADS],
                            start=(t == 0), stop=(t == ST_TILES - 1))

                    # epilogue: H = relu(U*(z>0)/max(z,eps) - (z>0)*fd)
                    z16 = eb.tile([P, HEADS], FP, tag="z16")
                    nc.scalar.activation(out=z16[:], in_=z_ps[:],
                                         func=mybir.ActivationFunctionType.Copy)
                    zm = eb.tile([P, HEADS], FP, tag="zm")
                    nc.vector.tensor_scalar(out=zm[:], in0=z16[:],
                                            scalar1=EPS_Z, scalar2=None,
                                            op0=mybir.AluOpType.max)
                    rz = eb.tile([P, HEADS], FP, tag="rz")
                    nc.vector.reciprocal(out=rz[:], in_=zm[:])
                    mk = eb.tile([P, HEADS], FP, tag="mk")
                    nc.vector.tensor_scalar(out=mk[:], in0=z16[:],
                                            scalar1=0.0, scalar2=None,
                                            op0=mybir.AluOpType.is_gt)
                    mrz = eb.tile([P, HEADS], FP, tag="mrz")
                    nc.vector.tensor_tensor(out=mrz[:], in0=rz[:], in1=mk[:],
                                            op=mybir.AluOpType.mult)
                    av = eb.tile([P, IN_FEAT], FP, tag="av")
                    nc.vector.tensor_tensor(
                        out=av[:].rearrange("p (hh d) -> p hh d", d=HEAD_DIM),
                        in0=U_ps[:].rearrange("p (hh d) -> p hh d",
                                              d=HEAD_DIM),
                        in1=mrz[:].unsqueeze(2).to_broadcast(
                            [P, HEADS, HEAD_DIM]),
                        op=mybir.AluOpType.mult)
                    fdm = eb.tile([P, IN_FEAT], FP, tag="fdm")
                    nc.vector.tensor_tensor(
                        out=fdm[:].rearrange("p (hh d) -> p hh d",
                                             d=HEAD_DIM),
                        in0=FDB[:].rearrange("p (hh d) -> p hh d",
                                             d=HEAD_DIM),
                        in1=mk[:].unsqueeze(2).to_broadcast(
                            [P, HEADS, HEAD_DIM]),
                        op=mybir.AluOpType.mult)
                    h2 = eb.tile([P, IN_FEAT], FP, tag="h2")
                    nc.gpsimd.tensor_tensor(out=h2[:], in0=av[:], in1=fdm[:],
                                            op=mybir.AluOpType.subtract)
                    ob = eb.tile([P, IN_FEAT], FP, tag="ob")
                    nc.scalar.activation(out=ob[:], in_=h2[:],
                                         func=mybir.ActivationFunctionType.Relu)
                    nc.sync.dma_start(
                        out=H_d[st * WIN:(st + 1) * WIN, :], in_=ob[:])
    nc.compile()
    return nc


_PROGRAM_CACHE = {}


def kernel(**inputs) -> np.ndarray:
    in_maps, metas, n_st, has_bias = _prep_cores(**inputs)
    key = (n_st, has_bias)
    if key not in _PROGRAM_CACHE:
        _PROGRAM_CACHE[key] = build_program(n_st, has_bias=has_bias)
    nc = _PROGRAM_CACHE[key]
    res = run_bass_kernel_spmd(nc, in_maps, list(range(N_CORES)))
    out = np.zeros((N_NODES, IN_FEAT), np.float32)
    for k in range(N_CORES):
        H = np.asarray(res.results[k]["H"])
        n0s = metas[k]
        base = k * N_LOCAL
        for st in range(n_st):
            n0 = int(n0s[st])
            hi = min(n0 + WIN, N_LOCAL)
            if hi > n0:
                out[base + n0:base + hi] += H[st * WIN:st * WIN + (hi - n0)]
    return out


# revision 12
# speedup vs baseline: 1.7798x; 1.7798x over previous
"""GATv2Conv-with-edge-features Trainium2 kernel (8-core SPMD, edge-sharded by dst).

Self-contained: hardcodes problem shapes (N=50000 nodes, E=800000 edges,
128 feat, 8 heads x 16). Sharding: core k owns dst nodes [6250k, 6250(k+1))
and all edges pointing into that range.

v3: block-aligned tiling, single-phase program.
Edges sorted by dst are packed into tiles of <=128 edges; a tile never
crosses a 32-node block boundary; a super-tile (st) is 16 tiles spanning
<=4 consecutive blocks, i.e. a 128-node window [n0, n0+128). Per st:
  one fp16 DMA [x[src] | efeat | x[window].T] + one fp8 DMA [S | S^T]
  FDwin(psum) = xwin.T @ Wd  -> fp16 (ACT copy): feat_dst for the window
  T(psum)[e,hd] = xs@Ws + ef@We + S^T @ FDwin   (3 matmuls/tile; the S^T
     one-hot broadcasts per-slot feat_dst to edges, slot = node - n0)
  T16 = ACT copy; L = max(T16, 0.2*T16); LA = L*attn (DVE fp16 2x);
  score = tree-add over d (fp16 2x, no TensorReduce); ex = exp(score);
  msg = T16*ex (Pool engine, both halves);
  U(psum)[slot,f] = sum_t S_t.T @ msg_t ; z[slot,h] likewise (slot rows of
     ALL 16 tiles accumulate into one [128,128] psum = window node rows)
  H = relu(U*(z>0)/max(z,eps) - (z>0)*FDwin)  -> stored to H_d[st*128 ...]
Host: out[n0:n0+128] += H[st] per st (non-owned rows are exactly 0), so the
program is identical across cores (SPMD) despite per-core windows.
"""
import numpy as np
import ml_dtypes

import concourse.bacc as bacc
import concourse.bass as bass
import concourse.tile as tile
import concourse.mybir as mybir
from concourse.bass_utils import run_bass_kernel_spmd

N_NODES = 50000
N_CORES = 8
N_LOCAL = N_NODES // N_CORES          # 6250
IN_FEAT = 128
HEADS = 8
HEAD_DIM = 16
NEG_SLOPE = 0.2
TILE_E = 128                          # edges per tile
BLK = 32                              # nodes per aligned block
ST_TILES = 16                         # tiles per super-tile
ST_BLKS = 4                           # max blocks spanned by one st
WIN = BLK * ST_BLKS                   # 128-node window per st
EDGE_BLK = ST_TILES * TILE_E          # 2048 edge slots per st
MG_COLS = 2 * EDGE_BLK + WIN          # xs | ef | xwinT = 4224 fp16 cols
OFF_XS = 0
OFF_EF = EDGE_BLK
OFF_XW = 2 * EDGE_BLK
OH_COLS = 2 * EDGE_BLK                # S | S^T = 4096 fp8 cols
EPS_Z = 1e-12
P = 128
FP = mybir.dt.float32
BF = mybir.dt.float16
F8 = mybir.dt.float8e4
BF_NP = np.float16
F8_NP = ml_dtypes.float8_e4m3fn


# ---------------------------------------------------------------- host prep

def _pack_core(dst_sorted_local, deg):
    """Block-aligned tile packing. Returns per-tile (block, count) and the
    edge order is implicit (dst-sorted). A tile's edges all lie in one
    32-node block; a node's edges never split across tiles."""
    tiles = []                  # (block, n_edges)
    cur_blk = -1
    cur_cnt = 0
    for n in range(len(deg)):
        d = int(deg[n])
        if d == 0:
            continue
        assert d <= TILE_E, f"node degree {d} exceeds tile capacity {TILE_E}"
        b = n // BLK
        if b != cur_blk or cur_cnt + d > TILE_E:
            if cur_cnt > 0:
                tiles.append((cur_blk, cur_cnt))
            cur_blk, cur_cnt = b, 0
        cur_cnt += d
    if cur_cnt > 0:
        tiles.append((cur_blk, cur_cnt))

    # group tiles into super-tiles: <=16 tiles, <=4 blocks span
    sts = []                    # list of (a0, [tile indices])
    cur = []
    a0 = None
    for ti, (b, _) in enumerate(tiles):
        if cur and (len(cur) == ST_TILES or b - a0 >= ST_BLKS):
            sts.append((a0, cur))
            cur = []
        if not cur:
            a0 = b
        cur.append(ti)
    if cur:
        sts.append((a0, cur))
    return tiles, sts


def _prep_cores(x, efeat, src, dst, W_src, b_src, W_dst, b_dst, W_edge, attn):
    x = np.ascontiguousarray(np.asarray(x, np.float32))
    efeat = np.asarray(efeat, np.float32)
    src = np.asarray(src).astype(np.int64)
    dst = np.asarray(dst).astype(np.int64)
    W_src = np.asarray(W_src, np.float32)
    W_dst = np.asarray(W_dst, np.float32)
    W_edge = np.asarray(W_edge, np.float32)
    b_src = np.asarray(b_src, np.float32)
    b_dst = np.asarray(b_dst, np.float32)
    attn = np.asarray(attn, np.float32)
    has_bias = bool(max(np.abs(b_src).max(), np.abs(b_dst).max()) > 0)

    per_core = []
    core_nst = []
    for k in range(N_CORES):
        lo = k * N_LOCAL
        eidx = np.nonzero((dst >= lo) & (dst < lo + N_LOCAL))[0]
        dl = dst[eidx] - lo
        order = np.argsort(dl, kind="stable")
        eidx, dl = eidx[order], dl[order]
        deg = np.bincount(dl, minlength=N_LOCAL)
        tiles, sts = _pack_core(dl, deg)
        per_core.append((eidx, dl, tiles, sts))
        core_nst.append(len(sts))

    n_st = max(core_nst)
    x16 = x.astype(BF_NP)
    ef16 = efeat.astype(BF_NP)
    attn16 = np.ascontiguousarray(
        np.broadcast_to(attn.reshape(1, HEADS * HEAD_DIM),
                        (P, HEADS * HEAD_DIM)).astype(BF_NP))
    bsrc16 = np.ascontiguousarray(
        np.broadcast_to(b_src[None, :], (P, IN_FEAT)).astype(BF_NP))
    bdst16 = np.ascontiguousarray(
        np.broadcast_to(b_dst[None, :], (P, IN_FEAT)).astype(BF_NP))

    in_maps = []
    metas = []
    for k in range(N_CORES):
        eidx, dl, tiles, sts = per_core[k]
        # tile edge start offsets
        starts = np.zeros(len(tiles) + 1, np.int64)
        for ti, (_, c) in enumerate(tiles):
            starts[ti + 1] = starts[ti] + c

        mg = np.zeros((P, n_st * MG_COLS), BF_NP)
        oh = np.zeros((P, n_st * OH_COLS), F8_NP)
        n0s = np.zeros(n_st, np.int64)
        xpad = np.zeros((WIN, IN_FEAT), BF_NP)
        for st, (a0, tl_list) in enumerate(sts):
            n0 = a0 * BLK
            n0s[st] = n0
            b0 = st * MG_COLS
            o0 = st * OH_COLS
            # window x rows (transposed) for on-device feat_dst projection
            hi = min(n0 + WIN, N_LOCAL)
            xw = xpad.copy()
            xw[:hi - n0] = x16[k * N_LOCAL + n0:k * N_LOCAL + hi]
            mg[:, b0 + OFF_XW:b0 + OFF_XW + WIN] = xw.T
            for tl, ti in enumerate(tl_list):
                e_ids = eidx[starts[ti]:starts[ti + 1]]
                slot = (dl[starts[ti]:starts[ti + 1]] - n0).astype(np.int64)
                c = len(e_ids)
                ep = np.arange(c)
                col = b0 + tl * TILE_E
                mg[:, col + OFF_XS:col + OFF_XS + c] = x16[src[e_ids]].T
                mg[:, col + OFF_EF:col + OFF_EF + c] = ef16[e_ids].T
                oh[ep, o0 + tl * TILE_E + slot] = 1.0          # S [e, slot]
                oh[slot, o0 + EDGE_BLK + tl * TILE_E + ep] = 1.0  # S^T
        in_maps.append(dict(
            Ws16=np.ascontiguousarray(W_src.T.astype(BF_NP)),
            Wd16=np.ascontiguousarray(W_dst.T.astype(BF_NP)),
            We16=np.ascontiguousarray(W_edge.T.astype(BF_NP)),
            bsrc16=bsrc16,
            bdst16=bdst16,
            attn16=attn16,
            mg=np.ascontiguousarray(mg),
            oh=np.ascontiguousarray(oh),
        ))
        metas.append(n0s)
    return in_maps, metas, n_st, has_bias


# ------------------------------------------------------------- bass program

def build_program(n_st, has_bias=False):
    nc = bacc.Bacc("TRN2", target_bir_lowering=False, debug=False,
                   num_devices=N_CORES)

    Ws16_d = nc.dram_tensor("Ws16", [IN_FEAT, IN_FEAT], BF, kind="ExternalInput")
    Wd16_d = nc.dram_tensor("Wd16", [IN_FEAT, IN_FEAT], BF, kind="ExternalInput")
    We16_d = nc.dram_tensor("We16", [IN_FEAT, IN_FEAT], BF, kind="ExternalInput")
    bsrc_d = nc.dram_tensor("bsrc16", [P, IN_FEAT], BF, kind="ExternalInput")
    bdst_d = nc.dram_tensor("bdst16", [P, IN_FEAT], BF, kind="ExternalInput")
    attn_d = nc.dram_tensor("attn16", [P, IN_FEAT], BF, kind="ExternalInput")
    mg_d = nc.dram_tensor("mg", [P, n_st * MG_COLS], BF, kind="ExternalInput")
    oh_d = nc.dram_tensor("oh", [P, n_st * OH_COLS], F8, kind="ExternalInput")
    H_d = nc.dram_tensor("H", [n_st * WIN, IN_FEAT], FP, kind="ExternalOutput")

    with tile.TileContext(nc) as tc:
        with tc.tile_pool(name="const", bufs=1) as cb:
            Ws16 = cb.tile([P, IN_FEAT], BF)
            nc.sync.dma_start(out=Ws16[:], in_=Ws16_d[:])
            Wd16 = cb.tile([P, IN_FEAT], BF)
            nc.sync.dma_start(out=Wd16[:], in_=Wd16_d[:])
            We16 = cb.tile([P, IN_FEAT], BF)
            nc.sync.dma_start(out=We16[:], in_=We16_d[:])
            bsrc = cb.tile([P, IN_FEAT], BF)
            nc.sync.dma_start(out=bsrc[:], in_=bsrc_d[:])
            bdst = cb.tile([P, IN_FEAT], BF)
            nc.sync.dma_start(out=bdst[:], in_=bdst_d[:])
            attn_sb = cb.tile([P, IN_FEAT], BF)
            nc.sync.dma_start(out=attn_sb[:], in_=attn_d[:])

            with (
                tc.tile_pool(name="eb_sb", bufs=3) as eb,
                tc.tile_pool(name="eb_ps", bufs=2, space="PSUM") as ep,
                tc.tile_pool(name="eb_psU", bufs=2, space="PSUM") as epU,
                tc.tile_pool(name="eb_psz", bufs=2, space="PSUM") as epz,
            ):
                ctx = {}

                def front(st):
                    b0 = st * MG_COLS
                    o0 = st * OH_COLS
                    mg_sb = eb.tile([P, MG_COLS], BF, tag="mg")
                    nc.sync.dma_start(out=mg_sb[:], in_=mg_d[:, b0:b0 + MG_COLS])
                    oh_sb = eb.tile([P, OH_COLS], F8, tag="oh")
                    nc.sync.dma_start(out=oh_sb[:], in_=oh_d[:, o0:o0 + OH_COLS])

                    # window feat_dst = xwin.T @ Wd  (on device)
                    f_ps = epU.tile([P, IN_FEAT], FP, tag="Ups")
                    nc.tensor.matmul(out=f_ps[:BLK, :1],
                                     lhsT=mg_sb[:, :BLK], rhs=mg_sb[:, :1],
                                     start=True, stop=True,
                                     skip_group_check=True)
                    nc.tensor.matmul(out=f_ps[:BLK, :1],
                                     lhsT=oh_sb[:, :BLK], rhs=oh_sb[:, :1],
                                     start=True, stop=True,
                                     skip_group_check=True)
                    nc.tensor.matmul(out=f_ps[:],
                                     lhsT=mg_sb[:, OFF_XW:OFF_XW + WIN],
                                     rhs=Wd16[:], start=True, stop=True)
                    FDB = eb.tile([P, IN_FEAT], BF, tag="FDB")
                    nc.scalar.activation(out=FDB[:], in_=f_ps[:],
                                         func=mybir.ActivationFunctionType.Copy)
                    if has_bias:
                        nc.vector.tensor_tensor(out=FDB[:], in0=FDB[:],
                                                in1=bdst[:],
                                                op=mybir.AluOpType.add)

                    T16 = []
                    for h in range(2):
                        Th = ep.tile([P, 8 * IN_FEAT], FP, tag="T")
                        for tt in range(8):
                            t = h * 8 + tt
                            c = t * TILE_E
                            sl = slice(tt * IN_FEAT, (tt + 1) * IN_FEAT)
                            nc.tensor.matmul(
                                out=Th[:, sl],
                                lhsT=mg_sb[:, OFF_XS + c:OFF_XS + c + TILE_E],
                                rhs=Ws16[:], start=True, stop=False)
                            nc.tensor.matmul(
                                out=Th[:, sl],
                                lhsT=mg_sb[:, OFF_EF + c:OFF_EF + c + TILE_E],
                                rhs=We16[:], start=False, stop=False)
                            nc.tensor.matmul(
                                out=Th[:, sl],
                                lhsT=oh_sb[:, EDGE_BLK + c:
                                           EDGE_BLK + c + TILE_E],
                                rhs=FDB[:], start=False, stop=True)
                        T16h = eb.tile([P, 8 * IN_FEAT], BF, tag=f"T16{h}")
                        nc.scalar.activation(out=T16h[:], in_=Th[:],
                                             func=mybir.ActivationFunctionType.Copy)
                        if has_bias:
                            nc.vector.tensor_tensor(
                                out=T16h[:].rearrange("p (t f) -> p t f", t=8),
                                in0=T16h[:].rearrange("p (t f) -> p t f", t=8),
                                in1=bsrc[:].unsqueeze(1).to_broadcast(
                                    [P, 8, IN_FEAT]),
                                op=mybir.AluOpType.add)
                        T16.append(T16h)

                    # score chain (DVE, fp16 2x where possible)
                    score = eb.tile([P, ST_TILES * HEADS], BF, tag="score")
                    for h in range(2):
                        T16h = T16[h]
                        T2 = eb.tile([P, 8 * IN_FEAT], BF, tag="T2")
                        nc.vector.tensor_scalar(
                            out=T2[:], in0=T16h[:], scalar1=NEG_SLOPE,
                            scalar2=None, op0=mybir.AluOpType.mult)
                        L = eb.tile([P, 8 * IN_FEAT], BF, tag="L")
                        nc.vector.tensor_tensor(out=L[:], in0=T16h[:],
                                                in1=T2[:],
                                                op=mybir.AluOpType.max)
                        LA = eb.tile([P, 8 * IN_FEAT], BF, tag="LA")
                        nc.vector.tensor_tensor(
                            out=LA[:].rearrange("p (t f) -> p t f", t=8),
                            in0=L[:].rearrange("p (t f) -> p t f", t=8),
                            in1=attn_sb[:].unsqueeze(1).to_broadcast(
                                [P, 8, IN_FEAT]),
                            op=mybir.AluOpType.mult)
                        LAv = LA[:].rearrange("p (t hh d) -> p t hh d",
                                              hh=HEADS, d=HEAD_DIM)
                        s1 = eb.tile([P, 512], BF, tag="s1")
                        s1v = s1[:].rearrange("p (t hh d) -> p t hh d",
                                              hh=HEADS, d=8)
                        nc.vector.tensor_tensor(out=s1v, in0=LAv[:, :, :, :8],
                                                in1=LAv[:, :, :, 8:],
                                                op=mybir.AluOpType.add)
                        s2 = eb.tile([P, 256], BF, tag="s2")
                        s2v = s2[:].rearrange("p (t hh d) -> p t hh d",
                                              hh=HEADS, d=4)
                        nc.vector.tensor_tensor(out=s2v, in0=s1v[:, :, :, :4],
                                                in1=s1v[:, :, :, 4:],
                                                op=mybir.AluOpType.add)
                        s3 = eb.tile([P, 128], BF, tag="s3")
                        s3v = s3[:].rearrange("p (t hh d) -> p t hh d",
                                              hh=HEADS, d=2)
                        nc.vector.tensor_tensor(out=s3v, in0=s2v[:, :, :, :2],
                                                in1=s2v[:, :, :, 2:],
                                                op=mybir.AluOpType.add)
                        sc = score[:, h * 64:(h + 1) * 64].rearrange(
                            "p (t hh) -> p t hh", hh=HEADS).unsqueeze(3)
                        nc.vector.tensor_tensor(out=sc, in0=s3v[:, :, :, :1],
                                                in1=s3v[:, :, :, 1:],
                                                op=mybir.AluOpType.add)

                    ex = eb.tile([P, ST_TILES * HEADS], BF, tag="ex")
                    nc.scalar.activation(out=ex[:], in_=score[:],
                                         func=mybir.ActivationFunctionType.Exp)

                    # msg = T16 * ex (broadcast over d) on Pool engine
                    msg = []
                    for h in range(2):
                        m = eb.tile([P, 8 * IN_FEAT], BF, tag=f"msg{h}")
                        ex_b = ex[:, h * 64:(h + 1) * 64] \
                            .rearrange("p (t hh) -> p t hh", hh=HEADS) \
                            .unsqueeze(3).to_broadcast([P, 8, HEADS, HEAD_DIM])
                        nc.gpsimd.tensor_tensor(
                            out=m[:].rearrange("p (t hh d) -> p t hh d",
                                               hh=HEADS, d=HEAD_DIM),
                            in0=T16[h][:].rearrange("p (t hh d) -> p t hh d",
                                                    hh=HEADS, d=HEAD_DIM),
                            in1=ex_b, op=mybir.AluOpType.mult)
                        msg.append(m)
                    ctx[st] = (oh_sb, FDB, ex, msg)

                def back(st):
                    oh_sb, FDB, ex, msg = ctx.pop(st)
                    # scatter all 16 tiles into one window psum
                    U_ps = epU.tile([P, IN_FEAT], FP, tag="Ups")
                    z_ps = epz.tile([P, HEADS], FP, tag="zps")
                    nc.tensor.matmul(out=U_ps[:BLK, :1],
                                     lhsT=msg[1][:, :BLK], rhs=msg[1][:, :1],
                                     start=True, stop=True,
                                     skip_group_check=True)
                    for t in range(ST_TILES):
                        h, tloc = t // 8, t % 8
                        Scol = oh_sb[:, t * TILE_E:(t + 1) * TILE_E]
                        nc.tensor.matmul(
                            out=U_ps[:],
                            lhsT=Scol,
                            rhs=msg[h][:, tloc * IN_FEAT:(tloc + 1) * IN_FEAT],
                            start=(t == 0), stop=(t == ST_TILES - 1))
                        nc.tensor.matmul(
                            out=z_ps[:],
                            lhsT=Scol,
                            rhs=ex[:, t * HEADS:(t + 1) * HEADS],
                            start=(t == 0), stop=(t == ST_TILES - 1))

                    # epilogue: H = relu(U/max(z,eps) - (z>0)*fd)
                    zm = eb.tile([P, HEADS], FP, tag="zm")
                    nc.vector.tensor_scalar(out=zm[:], in0=z_ps[:],
                                            scalar1=EPS_Z, scalar2=None,
                                            op0=mybir.AluOpType.max)
                    rz = eb.tile([P, HEADS], FP, tag="rz")
                    nc.vector.reciprocal(out=rz[:], in_=zm[:])
                    mk = eb.tile([P, HEADS], FP, tag="mk")
                    nc.vector.tensor_scalar(out=mk[:], in0=z_ps[:],
                                            scalar1=0.0, scalar2=None,
                                            op0=mybir.AluOpType.is_gt)
                    av = eb.tile([P, IN_FEAT], FP, tag="av")
                    nc.vector.tensor_tensor(
                        out=av[:].rearrange("p (hh d) -> p hh d", d=HEAD_DIM),
                        in0=U_ps[:].rearrange("p (hh d) -> p hh d",
                                              d=HEAD_DIM),
                        in1=rz[:].unsqueeze(2).to_broadcast(
                            [P, HEADS, HEAD_DIM]),
                        op=mybir.AluOpType.mult)
                    fdm = eb.tile([P, IN_FEAT], FP, tag="fdm")
                    nc.vector.tensor_tensor(
                        out=fdm[:].rearrange("p (hh d) -> p hh d",
                                             d=HEAD_DIM),
                        in0=FDB[:].rearrange("p (hh d) -> p hh d",
                                             d=HEAD_DIM),
                        in1=mk[:].unsqueeze(2).to_broadcast(
                            [P, HEADS, HEAD_DIM]),
                        op=mybir.AluOpType.mult)
                    h2 = eb.tile([P, IN_FEAT], FP, tag="h2")
                    nc.gpsimd.tensor_tensor(out=h2[:], in0=av[:], in1=fdm[:],
                                            op=mybir.AluOpType.subtract)
                    ob = eb.tile([P, IN_FEAT], FP, tag="ob")
                    nc.scalar.activation(out=ob[:], in_=h2[:],
                                         func=mybir.ActivationFunctionType.Relu)
                    nc.sync.dma_start(
                        out=H_d[st * WIN:(st + 1) * WIN, :], in_=ob[:])

                for st in range(n_st):
                    front(st)
                    if st >= 1:
                        back(st - 1)
                back(n_st - 1)
    nc.compile()
    return nc


_PROGRAM_CACHE = {}


def kernel(**inputs) -> np.ndarray:
    in_maps, metas, n_st, has_bias = _prep_cores(**inputs)
    key = (n_st, has_bias)
    if key not in _PROGRAM_CACHE:
        _PROGRAM_CACHE[key] = build_program(n_st, has_bias=has_bias)
    nc = _PROGRAM_CACHE[key]
    res = run_bass_kernel_spmd(nc, in_maps, list(range(N_CORES)))
    out = np.zeros((N_NODES, IN_FEAT), np.float32)
    for k in range(N_CORES):
        H = np.asarray(res.results[k]["H"])
        n0s = metas[k]
        base = k * N_LOCAL
        for st in range(n_st):
            n0 = int(n0s[st])
            hi = min(n0 + WIN, N_LOCAL)
            if hi > n0:
                out[base + n0:base + hi] += H[st * WIN:st * WIN + (hi - n0)]
    return out
